# revision 14
# baseline (speedup 1.0000x reference)
"""Shifted-window MSA (SWMSA) Trainium2 kernel — self-contained.

Contract: kernel(**inputs) takes the FULL unsharded inputs
  x [16, 16384, 512] f32, W_qkv [192,64], b_qkv [192], W_proj [512,512], b_proj [512]
and returns the full y [16, 16384, 512] f32, computed on 8 NeuronCores
(data-parallel over batch: 2 batches per core).

Algorithm notes (all exact, validated vs the reference):
- scoresT[k,q] = xk^T (Wq^T Wk / 8)^T xq + c_k; the q-only bias terms cancel
  in softmax over k; c_k is folded in via u' = u + Wk^T bq / 8 (per-partition
  bias on the u evacuation), since sum_e xT[e,k] wc[e] = c_k.
- v bias bv is deferred through the linear projection into the y bias.
- Softmax denominators come free via ones-columns in the attn@v matmul.
- Image rows/cols are loaded STRAIGHT (one big DMA per group); the
  reference's roll(-4,-4) is folded into the on-chip xT token-assembly
  copies (cols) and the DMA row indexing (rows). The output roll(+4,+4)
  is folded into the oT assembly (cols land pre-rolled so stores are
  straight) and the store row indexing.
- Scores are computed per-window (64-wide) so no cross-window garbage is
  ever exp'd; all 8 heads of a window-pair share one PSUM bank and a
  single [128,512] exp.
- dtype scheme: matmul moving operands are bf16 or f32r (N>=256), which
  the PE runs at 1 cycle/row; stationary operands keep higher precision
  where it is free (xT bf16, et f32r, Wp f32r).
"""

import numpy as np

import concourse.bass as bass
import concourse.bacc as bacc
import concourse.mybir as mybir
from concourse.tile import TileContext
from concourse.bass_utils import run_bass_kernel_spmd

F32 = mybir.dt.float32
F32R = mybir.dt.float32r
BF16 = mybir.dt.bfloat16
H = 128
NCORES = 8
PB = 2            # batches per core
SHIFT = 4
AF = mybir.ActivationFunctionType
MUL = mybir.AluOpType.mult

_CACHE = {}


def _make_consts(W_qkv, b_qkv, W_proj, b_proj):
    W_qkv = np.asarray(W_qkv, np.float64)
    b_qkv = np.asarray(b_qkv, np.float64)
    W_proj = np.asarray(W_proj, np.float64)
    b_proj = np.asarray(b_proj, np.float64)
    Wq, Wk, Wv = W_qkv[:64], W_qkv[64:128], W_qkv[128:192]
    bq, bv = b_qkv[:64], b_qkv[128:192]
    s = 1.0 / 8.0
    A = (Wq.T @ Wk) * s
    Mbd = np.zeros((128, 128), np.float32)
    Mbd[:64, :64] = A
    Mbd[64:, 64:] = A
    wc = np.tile((Wk.T @ bq) * s, 2).astype(np.float32).reshape(128, 1)
    WvTbd = np.zeros((128, 128), np.float32)
    WvTbd[:64, :64] = Wv.T
    WvTbd[64:, 64:] = Wv.T
    WpT = np.ascontiguousarray(W_proj.T.reshape(4, 128, 512)).astype(np.float32)
    by_row = b_proj + np.tile(bv, 8) @ W_proj.T
    by = np.broadcast_to(by_row.astype(np.float32), (128, 512)).copy()
    eye = np.eye(128, dtype=np.float32)
    return {"Mbd": Mbd, "wc": wc, "WvTbd": WvTbd.astype(np.float32),
            "WpT": WpT, "by": by, "eye": eye}


def _dram_ap(t, offset, dims):
    return bass.AP(tensor=t.tensor if isinstance(t, bass.AP) else t,
                   offset=offset, ap=[list(d) for d in dims])


def _sb_ap(tile, offset, free_dims, part=None):
    p = list(tile.ap[0])
    off = tile.offset + offset
    if part is not None:
        ps, pc = part
        off = off + ps * p[0]
        p = [p[0], pc]
    return bass.AP(tensor=tile.tensor, offset=off,
                   ap=[p] + [list(d) for d in free_dims])


def _runs(base, n, mod=H):
    """Split [base, base+n) mod `mod` into contiguous runs: (img_start, local_start, len)."""
    out, loc = [], 0
    while loc < n:
        st = (base + loc) % mod
        ln = min(n - loc, mod - st)
        out.append((st, loc, ln))
        loc += ln
    return out


def _build(nc, tc, PB=2, groups=None):
    if groups is None:
        groups = [(b, wh) for b in range(PB) for wh in range(16)]

    x = nc.dram_tensor("x", (PB, 16384, 512), F32R, kind="ExternalInput")
    y = nc.dram_tensor("y", (PB, 16384, 512), F32, kind="ExternalOutput")
    cMbd = nc.dram_tensor("Mbd", (128, 128), F32, kind="ExternalInput")
    cwc = nc.dram_tensor("wc", (128, 1), F32, kind="ExternalInput")
    cWv = nc.dram_tensor("WvTbd", (128, 128), F32, kind="ExternalInput")
    cWp = nc.dram_tensor("WpT", (4, 128, 512), F32R, kind="ExternalInput")
    cby = nc.dram_tensor("by", (128, 512), F32, kind="ExternalInput")
    ceye = nc.dram_tensor("eye", (128, 128), F32, kind="ExternalInput")

    with tc.tile_pool(name="consts", bufs=1) as cp, \
         tc.tile_pool(name="work", bufs=1) as wp, \
         tc.tile_pool(name="psum", bufs=1, space="PSUM") as pp:

        Mbd32 = cp.tile([128, 128], F32, name="cMbd32")
        nc.sync.dma_start(Mbd32[:], cMbd[:])
        Mbd = cp.tile([128, 128], BF16, name="cMbd")
        nc.vector.tensor_copy(Mbd[:], Mbd32[:])
        wc = cp.tile([128, 1], F32, name="cwc")
        nc.sync.dma_start(wc[:], cwc[:])
        Wv32 = cp.tile([128, 128], F32, name="cWv32")
        nc.sync.dma_start(Wv32[:], cWv[:])
        Wv = cp.tile([128, 128], BF16, name="cWv")
        nc.vector.tensor_copy(Wv[:], Wv32[:])
        Wp = []
        for i in range(4):
            t = cp.tile([128, 512], F32R, name=f"cWp{i}")
            nc.sync.dma_start(t[:], cWp[i])
            Wp.append(t)
        by = cp.tile([128, 512], F32, name="cby")
        nc.sync.dma_start(by[:], cby[:])
        eye32 = cp.tile([128, 128], F32, name="ceye32")
        nc.sync.dma_start(eye32[:], ceye[:])
        eyeb = cp.tile([128, 128], BF16, name="ceyeb")
        nc.vector.tensor_copy(eyeb[:], eye32[:])
        cones = cp.tile([128, 1], BF16, name="cones")
        nc.vector.memset(cones[:], 1.0)

        for gi, (b, wh) in enumerate(groups):
            _group(nc, wp, pp, x, y, b, wh, gi, Mbd, wc, Wv, Wp, by, eyeb, cones)


def _group(nc, wp, pp, x, y, b, wh, gi, Mbd, wc, Wv, Wp, by, eyeb, cones):
    rowruns = _runs(8 * wh + SHIFT, 8)     # (img_row, local_idx, len)

    # ---------- load x: rt [128 part = img col, (t, e)] straight ----------
    rt = wp.tile([128, 4096], F32R, tag="rt", bufs=3, name=f"rt{gi}")
    for (rs, t0, rl) in rowruns:
        nc.sync.dma_start(
            _sb_ap(rt, 512 * t0, [[512, rl], [1, 512]]),
            _dram_ap(x, (b * 16384 + rs * 128) * 512,
                     [[512, 128], [128 * 512, rl], [1, 512]]))

    # ---------- transpose -> xT [128 e-in-chunk, (ch, tok)] bf16 ----------
    # token pos within chunk = wwl*64 + t*8 + cm; img col = (8*wwl+cm+4)%128
    xT = wp.tile([128, 4096], BF16, tag="xT", bufs=3, name=f"xT{gi}")
    for t in range(8):
        tp = pp.tile([128, 512], F32R, tag="ab", bufs=2, name=f"tp{gi}_{t}")
        for ch in range(4):
            nc.tensor.transpose(tp[:, 128 * ch:128 * (ch + 1)],
                                rt[:, 512 * t + 128 * ch:512 * t + 128 * (ch + 1)],
                                eyeb[:])
        nc.vector.tensor_copy(
            _sb_ap(xT, 8 * t, [[1024, 4], [64, 15], [1, 8]]),
            _sb_ap(tp, 4, [[128, 4], [8, 15], [1, 8]]))
        nc.vector.tensor_copy(
            _sb_ap(xT, 8 * t + 960, [[1024, 4], [4, 2], [1, 4]]),
            _sb_ap(tp, 124, [[128, 4], [-124, 2], [1, 4]]))

    # ---------- u = (Wq^T Wk/8)^T-matmul, +wc via ACT bias ----------
    u_sb = wp.tile([128, 4096], BF16, tag="u", bufs=3, name=f"u{gi}")
    for p in range(4):
        for hf in range(2):
            c0 = 1024 * p + 512 * hf
            ups = pp.tile([128, 512], F32, tag="ab", bufs=2,
                          name=f"ups{gi}_{p}_{hf}")
            nc.tensor.matmul(ups[:], Mbd[:], xT[:, c0:c0 + 512],
                             start=True, stop=True)
            nc.scalar.activation(u_sb[:, c0:c0 + 512], ups[:], AF.Identity,
                                 bias=wc[:, 0:1])

    # ---------- v (token-major, straight [wp*512 + h*64 + vd]) ----------
    vt = wp.tile([128, 4096], BF16, tag="vt", bufs=3, name=f"vt{gi}")
    for p in range(4):
        for half in range(2):
            vps = pp.tile([128, 512], F32, tag="ab", bufs=2,
                          name=f"vps{gi}_{p}_{half}")
            for wq in range(4):
                wpair = 4 * half + wq
                nc.tensor.matmul(vps[:, 128 * wq:128 * (wq + 1)],
                                 xT[:, 1024 * p + 128 * wpair:1024 * p + 128 * wpair + 128],
                                 Wv[:], start=True, stop=True)
            nc.scalar.copy(
                _sb_ap(vt, 512 * 4 * half + 128 * p, [[512, 4], [1, 128]]),
                _sb_ap(vps, 0, [[128, 4], [1, 128]]))

    # ---------- per window-pair: scores, exp, attn@v, norm, proj ----------
    oT = wp.tile([128, 4096], BF16, tag="oT", bufs=3, name=f"oT{gi}")
    ysb = wp.tile([128, 4096], F32, tag="ysb", bufs=2, name=f"ysb{gi}")
    for wpair in range(8):
        # scores: per (head, window) 64x64 block; k on partitions (win0 rows
        # 0-63, win1 rows 64-127), q on free (col 64*h + q)
        sps = pp.tile([128, 512], F32, tag="sps", bufs=2, name=f"sps{gi}_{wpair}")
        ub = 512 * (wpair // 4) + 128 * (wpair % 4)
        for h in range(8):
            p, hr = h // 2, 64 * (h % 2)
            for win in range(2):
                nc.tensor.matmul(
                    sps[64 * win:64 * win + 64, 64 * h:64 * h + 64],
                    xT[hr:hr + 64, 1024 * p + 128 * wpair + 64 * win:
                       1024 * p + 128 * wpair + 64 * win + 64],
                    u_sb[hr:hr + 64, 1024 * p + ub + 64 * win:
                         1024 * p + ub + 64 * win + 64],
                    start=True, stop=True, tile_position=(hr, 64 * win))
        etw = wp.tile([128, 512], BF16, tag="et", bufs=8, name=f"et{gi}_{wpair}")
        nc.scalar.activation(etw[:], sps[:], AF.Exp)

        # attn@v into token-major psum [q, h*64 + vd]; softmax denominators
        # via separate N=1 ones-matmuls into dn [q, h]
        osb = wp.tile([128, 512], BF16, tag="osb", bufs=8, name=f"osb{gi}_{wpair}")
        rc = wp.tile([128, 8], F32, tag="rc", bufs=8, name=f"rc{gi}_{wpair}")
        ops = pp.tile([128, 512], F32, tag="ops", bufs=2,
                      name=f"ops{gi}_{wpair}")
        dn = pp.tile([128, 8], F32, tag="ops", bufs=2, name=f"dn{gi}_{wpair}")
        for h in range(8):
            for win in range(2):
                s64 = 64 * win
                nc.tensor.matmul(
                    ops[s64:s64 + 64, 64 * h:64 * h + 64],
                    etw[s64:s64 + 64, 64 * h:64 * h + 64],
                    vt[s64:s64 + 64, 512 * wpair + 64 * h:512 * wpair + 64 * h + 64],
                    start=True, stop=True, tile_position=(s64, s64))
                nc.tensor.matmul(
                    dn[s64:s64 + 64, h:h + 1],
                    etw[s64:s64 + 64, 64 * h:64 * h + 64],
                    cones[s64:s64 + 64, 0:1],
                    start=True, stop=True, tile_position=(s64, s64))
        nc.vector.reciprocal(rc[:], dn[:])
        nc.vector.tensor_mul(
            _sb_ap(osb, 0, [[64, 8], [1, 64]]),
            _sb_ap(ops, 0, [[64, 8], [1, 64]]),
            _sb_ap(rc, 0, [[1, 8], [0, 64]]))

        # transpose out (bf16) and assemble oT with the +4 column roll
        otp = pp.tile([128, 512], BF16, tag="otp", bufs=1, name=f"otp{gi}_{wpair}")
        for ch in range(4):
            nc.tensor.transpose(otp[:, 128 * ch:128 * (ch + 1)],
                                osb[:, 128 * ch:128 * (ch + 1)], eyeb[:])
        nc.scalar.copy(
            _sb_ap(oT, 128 * wpair + 4, [[1024, 4], [1, 124]]),
            _sb_ap(otp, 0, [[128, 4], [1, 124]]))
        nc.vector.tensor_copy(
            _sb_ap(oT, 128 * wpair, [[1024, 4], [1, 4]]),
            _sb_ap(otp, 124, [[128, 4], [1, 4]]))

        # projection: yps[img_col, f] accumulated over 4 e-chunks
        yps = pp.tile([128, 512], F32, tag="yps", bufs=1, name=f"yps{gi}_{wpair}")
        for ch in range(4):
            nc.tensor.matmul(yps[:], oT[:, 1024 * ch + 128 * wpair:
                                         1024 * ch + 128 * wpair + 128],
                             Wp[ch][:], start=(ch == 0), stop=(ch == 3))
        nc.vector.tensor_add(ysb[:, 512 * wpair:512 * wpair + 512], yps[:], by[:])

    # ---------- store: rows (8wh + wp + 4)%128, straight cols ----------
    for (rs, w0, rl) in rowruns:
        nc.gpsimd.dma_start(
            _dram_ap(y, (b * 16384 + rs * 128) * 512,
                     [[512, 128], [128 * 512, rl], [1, 512]]),
            _sb_ap(ysb, 512 * w0, [[512, rl], [1, 512]]))


def _get_nc():
    if "nc" not in _CACHE:
        nc = bacc.Bacc("TRN2", target_bir_lowering=False)
        with TileContext(nc) as tc:
            _build(nc, tc, PB=PB)
        nc.finalize()
        _CACHE["nc"] = nc
    return _CACHE["nc"]


def kernel(x, W_qkv, b_qkv, W_proj, b_proj):
    x = np.ascontiguousarray(np.asarray(x, np.float32))
    consts = _make_consts(W_qkv, b_qkv, W_proj, b_proj)
    nc = _get_nc()
    in_maps = []
    for c in range(NCORES):
        m = {"x": x[c * PB:(c + 1) * PB]}
        m.update(consts)
        in_maps.append(m)
    res = run_bass_kernel_spmd(nc, in_maps, core_ids=list(range(NCORES)))
    y = np.concatenate([res.results[c]["y"] for c in range(NCORES)], axis=0)
    return y.astype(np.float32)


# revision 15
# speedup vs baseline: 1.1770x; 1.1770x over previous
"""Shifted-window MSA (SWMSA) Trainium2 kernel — self-contained.

Contract: kernel(**inputs) takes the FULL unsharded inputs
  x [16, 16384, 512] f32, W_qkv [192,64], b_qkv [192], W_proj [512,512], b_proj [512]
and returns the full y [16, 16384, 512] f32, computed on 8 NeuronCores
(data-parallel over batch: 2 batches per core).

Algorithm notes (all exact, validated vs the reference):
- scoresT[k,q] = xk^T (Wq^T Wk / 8)^T xq + c_k; the q-only bias terms cancel
  in softmax over k; c_k is folded in via u' = u + Wk^T bq / 8 (per-partition
  bias on the u evacuation), since sum_e xT[e,k] wc[e] = c_k.
- v bias bv is deferred through the linear projection into the y bias.
- Softmax denominators come free via ones-columns in the attn@v matmul.
- Image rows/cols are loaded STRAIGHT (one big DMA per group); the
  reference's roll(-4,-4) is folded into the on-chip xT token-assembly
  copies (cols) and the DMA row indexing (rows). The output roll(+4,+4)
  is folded into the oT assembly (cols land pre-rolled so stores are
  straight) and the store row indexing.
- Scores are computed per-window (64-wide) so no cross-window garbage is
  ever exp'd; all 8 heads of a window-pair share one PSUM bank and a
  single [128,512] exp.
- dtype scheme: matmul moving operands are bf16 or f32r (N>=256), which
  the PE runs at 1 cycle/row; stationary operands keep higher precision
  where it is free (xT bf16, et f32r, Wp f32r).
"""

import numpy as np

import concourse.bass as bass
import concourse.bacc as bacc
import concourse.mybir as mybir
from concourse.tile import TileContext
from concourse.bass_utils import run_bass_kernel_spmd

F32 = mybir.dt.float32
F32R = mybir.dt.float32r
BF16 = mybir.dt.bfloat16
H = 128
NCORES = 8
PB = 2            # batches per core
SHIFT = 4
AF = mybir.ActivationFunctionType
MUL = mybir.AluOpType.mult

_CACHE = {}


def _make_consts(W_qkv, b_qkv, W_proj, b_proj):
    W_qkv = np.asarray(W_qkv, np.float64)
    b_qkv = np.asarray(b_qkv, np.float64)
    W_proj = np.asarray(W_proj, np.float64)
    b_proj = np.asarray(b_proj, np.float64)
    Wq, Wk, Wv = W_qkv[:64], W_qkv[64:128], W_qkv[128:192]
    bq, bv = b_qkv[:64], b_qkv[128:192]
    s = 1.0 / 8.0
    A = (Wq.T @ Wk) * s
    Mbd = np.zeros((128, 128), np.float32)
    Mbd[:64, :64] = A
    Mbd[64:, 64:] = A
    wc = np.tile((Wk.T @ bq) * s, 2).astype(np.float32).reshape(128, 1)
    WvTbd = np.zeros((128, 128), np.float32)
    WvTbd[:64, :64] = Wv.T
    WvTbd[64:, 64:] = Wv.T
    WpT = np.ascontiguousarray(W_proj.T.reshape(4, 128, 512)).astype(np.float32)
    by_row = b_proj + np.tile(bv, 8) @ W_proj.T
    by = np.broadcast_to(by_row.astype(np.float32), (128, 512)).copy()
    eye = np.eye(128, dtype=np.float32)
    return {"Mbd": Mbd, "wc": wc, "WvTbd": WvTbd.astype(np.float32),
            "WpT": WpT, "by": by, "eye": eye}


def _dram_ap(t, offset, dims):
    return bass.AP(tensor=t.tensor if isinstance(t, bass.AP) else t,
                   offset=offset, ap=[list(d) for d in dims])


def _sb_ap(tile, offset, free_dims, part=None):
    p = list(tile.ap[0])
    off = tile.offset + offset
    if part is not None:
        ps, pc = part
        off = off + ps * p[0]
        p = [p[0], pc]
    return bass.AP(tensor=tile.tensor, offset=off,
                   ap=[p] + [list(d) for d in free_dims])


def _runs(base, n, mod=H):
    """Split [base, base+n) mod `mod` into contiguous runs: (img_start, local_start, len)."""
    out, loc = [], 0
    while loc < n:
        st = (base + loc) % mod
        ln = min(n - loc, mod - st)
        out.append((st, loc, ln))
        loc += ln
    return out


def _build(nc, tc, PB=2, groups=None):
    if groups is None:
        groups = [(b, wh) for b in range(PB) for wh in range(16)]

    x = nc.dram_tensor("x", (PB, 16384, 512), F32R, kind="ExternalInput")
    y = nc.dram_tensor("y", (PB, 16384, 512), F32, kind="ExternalOutput")
    cMbd = nc.dram_tensor("Mbd", (128, 128), F32, kind="ExternalInput")
    cwc = nc.dram_tensor("wc", (128, 1), F32, kind="ExternalInput")
    cWv = nc.dram_tensor("WvTbd", (128, 128), F32, kind="ExternalInput")
    cWp = nc.dram_tensor("WpT", (4, 128, 512), F32R, kind="ExternalInput")
    cby = nc.dram_tensor("by", (128, 512), F32, kind="ExternalInput")
    ceye = nc.dram_tensor("eye", (128, 128), F32, kind="ExternalInput")

    with tc.tile_pool(name="consts", bufs=1) as cp, \
         tc.tile_pool(name="work", bufs=1) as wp, \
         tc.tile_pool(name="psum", bufs=1, space="PSUM") as pp:

        Mbd32 = cp.tile([128, 128], F32, name="cMbd32")
        nc.sync.dma_start(Mbd32[:], cMbd[:])
        Mbd = cp.tile([128, 128], BF16, name="cMbd")
        nc.vector.tensor_copy(Mbd[:], Mbd32[:])
        wc = cp.tile([128, 1], F32, name="cwc")
        nc.sync.dma_start(wc[:], cwc[:])
        Wv32 = cp.tile([128, 128], F32, name="cWv32")
        nc.sync.dma_start(Wv32[:], cWv[:])
        Wv = cp.tile([128, 128], BF16, name="cWv")
        nc.vector.tensor_copy(Wv[:], Wv32[:])
        Wp = []
        for i in range(4):
            t = cp.tile([128, 512], F32R, name=f"cWp{i}")
            nc.sync.dma_start(t[:], cWp[i])
            Wp.append(t)
        by = cp.tile([128, 512], F32, name="cby")
        nc.sync.dma_start(by[:], cby[:])
        eye32 = cp.tile([128, 128], F32, name="ceye32")
        nc.sync.dma_start(eye32[:], ceye[:])
        eyeb = cp.tile([128, 128], BF16, name="ceyeb")
        nc.vector.tensor_copy(eyeb[:], eye32[:])
        cones = cp.tile([128, 1], BF16, name="cones")
        nc.vector.memset(cones[:], 1.0)

        for gi, (b, wh) in enumerate(groups):
            _group(nc, wp, pp, x, y, b, wh, gi, Mbd, wc, Wv, Wp, by, eyeb, cones)


def _group(nc, wp, pp, x, y, b, wh, gi, Mbd, wc, Wv, Wp, by, eyeb, cones):
    rowruns = _runs(8 * wh + SHIFT, 8)     # (img_row, local_idx, len)

    # ---------- load x: rt [128 part = img col, (t, e)] straight ----------
    rt = wp.tile([128, 4096], F32R, tag="rt", bufs=3, name=f"rt{gi}")
    for (rs, t0, rl) in rowruns:
        nc.sync.dma_start(
            _sb_ap(rt, 512 * t0, [[512, rl], [1, 512]]),
            _dram_ap(x, (b * 16384 + rs * 128) * 512,
                     [[512, 128], [128 * 512, rl], [1, 512]]))

    # ---------- transpose -> xT [128 e-in-chunk, (ch, tok)] bf16 ----------
    # token pos within chunk = wwl*64 + t*8 + cm; img col = (8*wwl+cm+4)%128
    xT = wp.tile([128, 4096], BF16, tag="xT", bufs=3, name=f"xT{gi}")
    for t in range(8):
        tp = pp.tile([128, 512], F32R, tag="tps", bufs=1, name=f"tp{gi}_{t}")
        for ch in range(4):
            nc.tensor.transpose(tp[:, 128 * ch:128 * (ch + 1)],
                                rt[:, 512 * t + 128 * ch:512 * t + 128 * (ch + 1)],
                                eyeb[:])
        nc.vector.tensor_copy(
            _sb_ap(xT, 8 * t, [[1024, 4], [64, 15], [1, 8]]),
            _sb_ap(tp, 4, [[128, 4], [8, 15], [1, 8]]))
        nc.vector.tensor_copy(
            _sb_ap(xT, 8 * t + 960, [[1024, 4], [4, 2], [1, 4]]),
            _sb_ap(tp, 124, [[128, 4], [-124, 2], [1, 4]]))

    # ---------- u = (Wq^T Wk/8)^T-matmul, +wc via ACT bias ----------
    u_sb = wp.tile([128, 4096], BF16, tag="u", bufs=3, name=f"u{gi}")
    for p in range(4):
        for hf in range(2):
            c0 = 1024 * p + 512 * hf
            ups = pp.tile([128, 512], F32, tag="uv", bufs=2,
                          name=f"ups{gi}_{p}_{hf}")
            nc.tensor.matmul(ups[:], Mbd[:], xT[:, c0:c0 + 512],
                             start=True, stop=True)
            nc.scalar.activation(u_sb[:, c0:c0 + 512], ups[:], AF.Identity,
                                 bias=wc[:, 0:1])

    # ---------- v (token-major, straight [wp*512 + h*64 + vd]) ----------
    vt = wp.tile([128, 4096], BF16, tag="vt", bufs=3, name=f"vt{gi}")
    for p in range(4):
        for half in range(2):
            vps = pp.tile([128, 512], F32, tag="uv", bufs=2,
                          name=f"vps{gi}_{p}_{half}")
            for wq in range(4):
                wpair = 4 * half + wq
                nc.tensor.matmul(vps[:, 128 * wq:128 * (wq + 1)],
                                 xT[:, 1024 * p + 128 * wpair:1024 * p + 128 * wpair + 128],
                                 Wv[:], start=True, stop=True)
            nc.scalar.copy(
                _sb_ap(vt, 512 * 4 * half + 128 * p, [[512, 4], [1, 128]]),
                _sb_ap(vps, 0, [[128, 4], [1, 128]]))

    # ---------- per window-pair: scores, exp, attn@v, norm, proj ----------
    oT = wp.tile([128, 4096], BF16, tag="oT", bufs=3, name=f"oT{gi}")
    ysb = wp.tile([128, 4096], F32, tag="ysb", bufs=2, name=f"ysb{gi}")
    for wpair in range(8):
        # scores: per (head, window) 64x64 block; k on partitions (win0 rows
        # 0-63, win1 rows 64-127), q on free (col 64*h + q)
        sps = pp.tile([128, 512], F32, tag="sdo", bufs=3, name=f"sps{gi}_{wpair}")
        ub = 512 * (wpair // 4) + 128 * (wpair % 4)
        for h in range(8):
            p, hr = h // 2, 64 * (h % 2)
            for win in range(2):
                nc.tensor.matmul(
                    sps[64 * win:64 * win + 64, 64 * h:64 * h + 64],
                    xT[hr:hr + 64, 1024 * p + 128 * wpair + 64 * win:
                       1024 * p + 128 * wpair + 64 * win + 64],
                    u_sb[hr:hr + 64, 1024 * p + ub + 64 * win:
                         1024 * p + ub + 64 * win + 64],
                    start=True, stop=True, tile_position=(hr, 64 * win))
        etw = wp.tile([128, 512], BF16, tag="et", bufs=8, name=f"et{gi}_{wpair}")
        nc.scalar.activation(etw[:], sps[:], AF.Exp)

        # attn@v into token-major psum [q, h*64 + vd]; softmax denominators
        # via separate N=1 ones-matmuls into dn [q, h]
        osb = wp.tile([128, 512], BF16, tag="osb", bufs=8, name=f"osb{gi}_{wpair}")
        rc = wp.tile([128, 8], F32, tag="rc", bufs=8, name=f"rc{gi}_{wpair}")
        ops = pp.tile([128, 512], F32, tag="sdo", bufs=3,
                      name=f"ops{gi}_{wpair}")
        dn = pp.tile([128, 8], F32, tag="sdo", bufs=3, name=f"dn{gi}_{wpair}")
        for h in range(8):
            for win in range(2):
                s64 = 64 * win
                nc.tensor.matmul(
                    ops[s64:s64 + 64, 64 * h:64 * h + 64],
                    etw[s64:s64 + 64, 64 * h:64 * h + 64],
                    vt[s64:s64 + 64, 512 * wpair + 64 * h:512 * wpair + 64 * h + 64],
                    start=True, stop=True, tile_position=(s64, s64))
                nc.tensor.matmul(
                    dn[s64:s64 + 64, h:h + 1],
                    etw[s64:s64 + 64, 64 * h:64 * h + 64],
                    cones[s64:s64 + 64, 0:1],
                    start=True, stop=True, tile_position=(s64, s64))
        nc.vector.reciprocal(rc[:], dn[:])
        nc.vector.tensor_mul(
            _sb_ap(osb, 0, [[64, 8], [1, 64]]),
            _sb_ap(ops, 0, [[64, 8], [1, 64]]),
            _sb_ap(rc, 0, [[1, 8], [0, 64]]))

        # transpose out (bf16) and assemble oT with the +4 column roll
        otp = pp.tile([128, 512], BF16, tag="oy", bufs=2, name=f"otp{gi}_{wpair}")
        for ch in range(4):
            nc.tensor.transpose(otp[:, 128 * ch:128 * (ch + 1)],
                                osb[:, 128 * ch:128 * (ch + 1)], eyeb[:])
        nc.scalar.copy(
            _sb_ap(oT, 128 * wpair + 4, [[1024, 4], [1, 124]]),
            _sb_ap(otp, 0, [[128, 4], [1, 124]]))
        nc.vector.tensor_copy(
            _sb_ap(oT, 128 * wpair, [[1024, 4], [1, 4]]),
            _sb_ap(otp, 124, [[128, 4], [1, 4]]))

        # projection: yps[img_col, f] accumulated over 4 e-chunks
        yps = pp.tile([128, 512], F32, tag="oy", bufs=2, name=f"yps{gi}_{wpair}")
        for ch in range(4):
            nc.tensor.matmul(yps[:], oT[:, 1024 * ch + 128 * wpair:
                                         1024 * ch + 128 * wpair + 128],
                             Wp[ch][:], start=(ch == 0), stop=(ch == 3))
        nc.vector.tensor_add(ysb[:, 512 * wpair:512 * wpair + 512], yps[:], by[:])

    # ---------- store: rows (8wh + wp + 4)%128, straight cols ----------
    for (rs, w0, rl) in rowruns:
        nc.gpsimd.dma_start(
            _dram_ap(y, (b * 16384 + rs * 128) * 512,
                     [[512, 128], [128 * 512, rl], [1, 512]]),
            _sb_ap(ysb, 512 * w0, [[512, rl], [1, 512]]))


def _get_nc():
    if "nc" not in _CACHE:
        nc = bacc.Bacc("TRN2", target_bir_lowering=False)
        with TileContext(nc) as tc:
            _build(nc, tc, PB=PB)
        nc.finalize()
        _CACHE["nc"] = nc
    return _CACHE["nc"]


def kernel(x, W_qkv, b_qkv, W_proj, b_proj):
    x = np.ascontiguousarray(np.asarray(x, np.float32))
    consts = _make_consts(W_qkv, b_qkv, W_proj, b_proj)
    nc = _get_nc()
    in_maps = []
    for c in range(NCORES):
        m = {"x": x[c * PB:(c + 1) * PB]}
        m.update(consts)
        in_maps.append(m)
    res = run_bass_kernel_spmd(nc, in_maps, core_ids=list(range(NCORES)))
    y = np.concatenate([res.results[c]["y"] for c in range(NCORES)], axis=0)
    return y.astype(np.float32)
